# revision 1
# baseline (speedup 1.0000x reference)
"""Benes butterfly network (12 layers, N=4096) on 8 Trainium2 NeuronCores.

Self-contained: takes full inputs, shards batch across 8 cores, runs a
Bass/Tile kernel per core, gathers the full output.

Math: reference layer k is a butterfly with span 2^k:
    h[:, j] <- A_k[j] * h[:, j] + B_k[j] * h[:, j ^ 2^k]
(A_k/B_k extracted from the sparse COO (values, idx_in, idx_out)).

Device decomposition per core (batch shard 512, transposed layout
[col-part, batch-free], 32 col-tiles of 128; x is pre-transposed on the
host so H0 tiles stream in with perfectly coalesced DMA):
  1. phase1: layers 0..8 fused into dense 128x128 block matrices, with
     layer 9's self-scale A9 folded in on the host:
       p1'[t] = sum_{j=0..3} (diag(A9[t]) @ M9[t, t^j]) @ H0[t^j]   (fp32r)
  2. L9 partner via ratio trick: H9[t] = E[t] + (B9[t]/A9[t^4]) * E[t^4]
     where E[t] = evac(p1'[t]) — one ACT/DVE copy + one DVE stt per tile.
  3. L10+L11 (dists 8, 16) fused into the PE out-transpose: for each
     quad {q, q+8, q+16, q+24}: psum[b, 4*128] accumulates 4 matmuls
     stationary=H9[src] b-block, moving=[diag(c[d0<-s])|...] (host-built).
  4. Evacuate pieces + strided DMA back to DRAM rows.
"""
import os
import numpy as np

N = 4096
BATCH = 4096
NLAYERS = 12
NCORES = 8
BSH = BATCH // NCORES      # 512 batch rows per core
T = N // 128               # 32 column tiles

_PROGRAM_CACHE = {}
LAST_EXEC_NS = None


def _extract_ab(values, idx_in, idx_out):
    """Per-layer butterfly coefficients A[k], B[k] (float64 [L, N])."""
    v = np.asarray(values, np.float64)
    ii = np.asarray(idx_in, np.int64)
    io = np.asarray(idx_out, np.int64)
    L, nnz = v.shape
    n = nnz // 2
    A = np.zeros((L, n))
    B = np.zeros((L, n))
    for k in range(L):
        s = 1 << k
        self_m = ii[k] == io[k]
        part_m = ii[k] == (io[k] ^ s)
        if not np.all(self_m | part_m):
            raise ValueError(f"layer {k}: unexpected sparse index structure")
        np.add.at(A[k], io[k][self_m], v[k][self_m])
        np.add.at(B[k], io[k][part_m], v[k][part_m])
    return A, B


def _host_precompute(values, idx_in, idx_out):
    A, B = _extract_ab(values, idx_in, idx_out)
    Ab = A.reshape(NLAYERS, T, 128)
    Bb = B.reshape(NLAYERS, T, 128)
    j = np.arange(128)

    # Block-level composition of layers 0..8: S[t] = {src_tile: 128x128}.
    S = [{t: np.eye(128)} for t in range(T)]
    for k in range(7):  # within-block layers
        s = 1 << k
        for t in range(T):
            W = np.zeros((128, 128))
            W[j, j] = Ab[k, t]
            W[j, j ^ s] = Bb[k, t]
            S[t] = {src: W @ M for src, M in S[t].items()}
    for k in (7, 8):   # cross-block layers, tile distance d
        d = 1 << (k - 7)
        newS = []
        for t in range(T):
            out = {}
            for src, M in S[t].items():
                out[src] = Ab[k, t][:, None] * M
            for src, M in S[t ^ d].items():
                out[src] = out.get(src, 0) + Bb[k, t][:, None] * M
            newS.append(out)
        S = newS

    # fold layer-9 self scale; guard against pathological tiny A9
    A9 = Ab[9].copy()
    tiny = np.abs(A9) < 1e-12
    if tiny.any():
        A9 = np.where(tiny, 1e-12, A9)
    mst = np.zeros((128, T * 512), np.float32)
    for t in range(T):
        assert set(S[t].keys()) == {t, t ^ 1, t ^ 2, t ^ 3}
        for ji in range(4):
            M = A9[t][:, None] * S[t][t ^ ji]
            mst[:, t * 512 + ji * 128: t * 512 + (ji + 1) * 128] = (
                M.T.astype(np.float32)
            )

    # L9 partner ratio scales rB9[t] = B9[t] / A9[t^4]
    scales = np.zeros((128, 32), np.float32)
    for t in range(T):
        scales[:, t] = (Bb[9, t] / A9[t ^ 4]).astype(np.float32)

    # out-transpose movings, quad-major: for quad q, slot si (src s=q+8*si),
    # block k holds diag(c[q+8k <- s]) where c are the fused L10*L11
    # coefficients acting on H9 (post-L9 state)
    movd = np.zeros((128, T * 512), np.float32)
    for s in range(T):
        q = s & 7
        si = s >> 3
        for k in range(4):
            d = q + 8 * k
            if s == d:
                c = Ab[11, d] * Ab[10, d]
            elif s == (d ^ 8):
                c = Ab[11, d] * Bb[10, d]
            elif s == (d ^ 16):
                c = Bb[11, d] * Ab[10, d ^ 16]
            else:  # s == d ^ 24
                c = Bb[11, d] * Bb[10, d ^ 16]
            movd[j, q * 2048 + si * 512 + k * 128 + j] = c.astype(np.float32)
    return mst, scales, movd


def _build_program():
    import concourse.bass as bass
    import concourse.mybir as mybir
    import concourse.tile as tile
    from concourse import bacc

    f32 = mybir.dt.float32
    f32r = mybir.dt.float32r
    mult = mybir.AluOpType.mult
    add = mybir.AluOpType.add

    nc = bacc.Bacc("TRN2", target_bir_lowering=False, debug=False)
    # x pre-transposed on host: [N, BSH] (column-major over batch shard)
    xT_ap = nc.dram_tensor("xT", [N, BSH], f32r, kind="ExternalInput").ap()
    mst_ap = nc.dram_tensor("mst", [128, T * 512], f32r, kind="ExternalInput").ap()
    sc_ap = nc.dram_tensor("scales", [128, 32], f32, kind="ExternalInput").ap()
    mov_ap = nc.dram_tensor("movd", [128, T * 512], f32r, kind="ExternalInput").ap()
    out_ap = nc.dram_tensor("out", [BSH, N], f32, kind="ExternalOutput").ap()

    with tile.TileContext(nc) as tc:
        with (
            tc.tile_pool(name="const", bufs=1) as constp,
            tc.tile_pool(name="h0", bufs=8) as h0p,
            tc.tile_pool(name="mstp", bufs=6) as mstp,
            tc.tile_pool(name="H", bufs=40) as Hp,
            tc.tile_pool(name="mov", bufs=4) as movp,
            tc.tile_pool(name="piece", bufs=6) as piecep,
            tc.tile_pool(name="ps", bufs=8, space="PSUM") as psp,
        ):
            # first mst chunk leads the ACT ring so phase1 starts early
            msts = {}
            msts[0] = mstp.tile([128, 512], f32r, tag="mst", name="mst_c0")
            nc.scalar.dma_start(msts[0][:], mst_ap[:, 0:512])
            sc = constp.tile([128, 32], f32)
            nc.scalar.dma_start(sc[:], sc_ap[:])

            # H0 tiles via 1MB 3D-strided DMAs, resident for all 4 passes:
            # H0cat[kb][p, lt*512+b] = xT[(4*kb+lt)*128 + p, b]
            H0cat = {}
            for kb in range(8):
                h0c = h0p.tile([128, 2048], f32r, tag="h0", name=f"h0c_{kb}")
                src = xT_ap[kb * 512:(kb + 1) * 512, :].rearrange(
                    "(lt p) b -> p lt b", lt=4, p=128
                )
                nc.sync.dma_start(h0c[:].rearrange("p (lt b) -> p lt b", lt=4), src)
                H0cat[kb] = h0c

            E, H9 = {}, {}
            for qt in range(8):
                for lt in range(4):
                    t = 4 * qt + lt
                    if t not in msts:
                        msts[t] = mstp.tile(
                            [128, 512], f32r, tag="mst", name=f"mst_c{t}"
                        )
                        nc.scalar.dma_start(
                            msts[t][:], mst_ap[:, t * 512:(t + 1) * 512]
                        )
                    mchunk = msts[t]
                    p1 = psp.tile([128, 512], f32, name=f"p1_{t}", tag="ps")
                    for ji in range(4):
                        nc.tensor.matmul(
                            p1[:],
                            mchunk[:, ji * 128:(ji + 1) * 128],
                            H0cat[qt][:, (lt ^ ji) * 512:((lt ^ ji) + 1) * 512],
                            start=(ji == 0), stop=(ji == 3),
                        )
                    E[t] = Hp.tile([128, 512], f32r, tag="H", name=f"E_{t}")
                    if t % 2 == 0:
                        nc.scalar.copy(E[t][:], p1[:])
                    else:
                        nc.vector.tensor_copy(E[t][:], p1[:])
                if qt % 2 == 1:
                    # L9 for the finished 8-group: H9[t] = E[t] + rB9[t]*E[t^4]
                    g = qt // 2
                    for t in range(8 * g, 8 * g + 8):
                        H9[t] = Hp.tile([128, 512], f32r, tag="H", name=f"H9_{t}")
                        nc.vector.scalar_tensor_tensor(
                            H9[t][:], E[t ^ 4][:], sc[:, t:t + 1], E[t][:],
                            op0=mult, op1=add,
                        )

            # ---- out-transpose + L10 + L11 (quads) ----
            for q in range(8):
                srcs = [q, q + 8, q + 16, q + 24]
                mv = movp.tile([128, 2048], f32r, tag="mov", name=f"mov_{q}")
                nc.scalar.dma_start(mv[:], mov_ap[:, q * 2048:(q + 1) * 2048])
                for bb in range(4):
                    pq = psp.tile([128, 512], f32, tag="ps", name=f"pq_{q}_{bb}")
                    for si, s in enumerate(srcs):
                        nc.tensor.matmul(
                            pq[:], H9[s][:, bb * 128:(bb + 1) * 128],
                            mv[:, si * 512:(si + 1) * 512],
                            start=(si == 0), stop=(si == 3),
                        )
                    piece = piecep.tile([128, 512], f32, tag="piece")
                    if (q + bb) % 2 == 0:
                        nc.scalar.copy(piece[:], pq[:])
                    else:
                        nc.vector.tensor_copy(piece[:], pq[:])
                    dst = out_ap[bb * 128:(bb + 1) * 128, :].rearrange(
                        "p (k t c) -> p k t c", k=4, t=8, c=128
                    )[:, :, q, :]
                    src = piece[:].rearrange("p (k c) -> p k c", k=4, c=128)
                    nc.sync.dma_start(dst, src)

    nc.compile()
    return nc


def kernel(x, values, idx_in, idx_out):
    global LAST_EXEC_NS
    from concourse.bass_utils import run_bass_kernel_spmd

    x = np.asarray(x, np.float32)
    assert x.shape == (BATCH, N), x.shape
    mst, scales, movd = _host_precompute(values, idx_in, idx_out)
    xT = np.ascontiguousarray(x.T)

    if "prog" not in _PROGRAM_CACHE:
        _PROGRAM_CACHE["prog"] = _build_program()
    nc = _PROGRAM_CACHE["prog"]

    in_maps = [
        {
            "xT": np.ascontiguousarray(xT[:, i * BSH:(i + 1) * BSH]),
            "mst": mst,
            "scales": scales,
            "movd": movd,
        }
        for i in range(NCORES)
    ]
    res = run_bass_kernel_spmd(nc, in_maps, core_ids=list(range(NCORES)))
    if os.environ.get("BENES_TRACE"):
        tres = run_bass_kernel_spmd(
            nc, in_maps, core_ids=list(range(NCORES)), trace=True
        )
        LAST_EXEC_NS = tres.exec_time_ns
        _PROGRAM_CACHE["profile_json"] = tres.profile_json
    out = np.empty((BATCH, N), np.float32)
    for i in range(NCORES):
        out[i * BSH:(i + 1) * BSH] = res.results[i]["out"]
    return out



# revision 7
# speedup vs baseline: 1.0788x; 1.0788x over previous
"""Benes butterfly network (12 layers, N=4096) on 8 Trainium2 NeuronCores.

Self-contained: takes full inputs, shards batch across 8 cores, runs a
Bass/Tile kernel per core, gathers the full output.

Math: reference layer k is a butterfly with span 2^k:
    h[:, j] <- A_k[j] * h[:, j] + B_k[j] * h[:, j ^ 2^k]
(A_k/B_k extracted from the sparse COO (values, idx_in, idx_out)).

v3 design (DMA-bound problem; per-core traffic cut 33.6MB -> ~17MB):
  - Everything on the wire is bf16 (host-cast); PSUM accumulates fp32.
  - Layers 0..8 composed on the host into dense 128x128 block matrices
    (mst, bf16), with layer 9's self-scale A9 AND the L10/L11 self-scales
    u[t] = A10[t]*A11[t] folded in:
      p1[t] = sum_{ji=0..3} M[t, t^ji] @ H0[t^ji]      (PE, N=512, fp32 psum)
  - L9/L10/L11 as three ratio passes (tile distances 4, 8, 16), exact
    algebra, scalars precomputed in float64:
      F[t]   = p1[t] + s9[t]  * p1[t^4]     (stt directly from PSUM)
      P[t]   = F[t] + s10[t] * F[t^8]       (stt, SBUF bf16)
      H11[t] = P[t] + s11[t] * P[t^16]      (tensor_scalar + tensor_tensor)
  - Phase 2 is a pure PE transpose: matmuls into 128-col PSUM quarters with
    a constant bf16 identity moving operand (no 8.4MB diag-matrix stream).
  - Residue pipelining: output tile d only depends on input tiles
    t = d (mod 4); emission is software-pipelined so the PE queue runs
    phase1(r+1) before phase2(r) and never stalls on the DVE chain.
"""
import os
import numpy as np
import ml_dtypes

N = 4096
BATCH = 4096
NLAYERS = 12
NCORES = 8
BSH = BATCH // NCORES      # 512 batch rows per core
T = N // 128               # 32 column tiles

_PROGRAM_CACHE = {}
LAST_EXEC_NS = None


def _extract_ab(values, idx_in, idx_out):
    """Per-layer butterfly coefficients A[k], B[k] (float64 [L, N])."""
    v = np.asarray(values, np.float64)
    ii = np.asarray(idx_in, np.int64)
    io = np.asarray(idx_out, np.int64)
    L, nnz = v.shape
    n = nnz // 2
    A = np.zeros((L, n))
    B = np.zeros((L, n))
    for k in range(L):
        s = 1 << k
        self_m = ii[k] == io[k]
        part_m = ii[k] == (io[k] ^ s)
        if not np.all(self_m | part_m):
            raise ValueError(f"layer {k}: unexpected sparse index structure")
        np.add.at(A[k], io[k][self_m], v[k][self_m])
        np.add.at(B[k], io[k][part_m], v[k][part_m])
    return A, B


def _clamp(a):
    return np.where(np.abs(a) < 1e-12, 1e-12, a)


# Residue-major tile ordering: seq[r*8 + m] = 4*m + r
_TSEQ = [4 * m + r for r in range(4) for m in range(8)]


def _host_precompute(values, idx_in, idx_out):
    A, B = _extract_ab(values, idx_in, idx_out)
    Ab = A.reshape(NLAYERS, T, 128)
    Bb = B.reshape(NLAYERS, T, 128)
    j = np.arange(128)

    # Dense composition of within-block layers 0..6, one 128x128 per tile.
    S = [np.eye(128) for _ in range(T)]
    for k in range(7):
        s = 1 << k
        for t in range(T):
            W = np.zeros((128, 128))
            W[j, j] = Ab[k, t]
            W[j, j ^ s] = Bb[k, t]
            S[t] = W @ S[t]
    # Cross-block layers 7, 8 (tile distances 1, 2): dict src_tile -> 128x128
    Sd = [{t: S[t]} for t in range(T)]
    for k in (7, 8):
        d = 1 << (k - 7)
        newS = []
        for t in range(T):
            out = {}
            for src, M in Sd[t].items():
                out[src] = Ab[k, t][:, None] * M
            for src, M in Sd[t ^ d].items():
                out[src] = out.get(src, 0) + Bb[k, t][:, None] * M
            newS.append(out)
        Sd = newS

    A9 = _clamp(Ab[9])
    A10 = _clamp(Ab[10])
    A11 = _clamp(Ab[11])
    u = A10 * A11  # folded into mst rows

    # mst (bf16, residue-major tile order): block (t, ji) holds
    # (diag(u[t]*A9[t]) @ Sd[t][t^ji]).T so matmul computes the row-scaled
    # M @ H0.
    mst = np.zeros((128, T * 512), np.float32)
    for si, t in enumerate(_TSEQ):
        assert set(Sd[t].keys()) == {t, t ^ 1, t ^ 2, t ^ 3}
        for ji in range(4):
            M = (u[t] * A9[t])[:, None] * Sd[t][t ^ ji]
            mst[:, si * 512 + ji * 128: si * 512 + (ji + 1) * 128] = (
                M.T.astype(np.float32)
            )

    # fp32 scalar tables: s9 | s10 | s11 (stt/ts scalars must be fp32)
    tabs = np.zeros((128, T), np.float64)
    tabs16 = np.zeros((128, 2 * T), np.float64)
    for t in range(T):
        tabs[:, t] = u[t] * (Bb[9, t] / A9[t ^ 4]) / u[t ^ 4]
        tabs16[:, t] = Ab[11, t] * Bb[10, t] / (A10[t ^ 8] * A11[t ^ 8])
        tabs16[:, T + t] = Bb[11, t] / A11[t ^ 16]

    ident = np.eye(128, dtype=np.float32)
    return (
        mst.astype(ml_dtypes.bfloat16),
        tabs.astype(np.float32),
        tabs16.astype(np.float32),
        ident.astype(ml_dtypes.bfloat16),
    )


def _build_program():
    import concourse.bass as bass
    import concourse.mybir as mybir
    import concourse.tile as tile
    from concourse import bacc

    f32 = mybir.dt.float32
    bf16 = mybir.dt.bfloat16
    mult = mybir.AluOpType.mult
    add = mybir.AluOpType.add

    nc = bacc.Bacc("TRN2", target_bir_lowering=False, debug=False)
    xT_ap = nc.dram_tensor("xT", [N, BSH], bf16, kind="ExternalInput").ap()
    mst_ap = nc.dram_tensor("mst", [128, T * 512], bf16, kind="ExternalInput").ap()
    tabs_ap = nc.dram_tensor("tabs", [128, T], f32, kind="ExternalInput").ap()
    tabs16_ap = nc.dram_tensor(
        "tabs16", [128, 2 * T], f32, kind="ExternalInput"
    ).ap()
    id_ap = nc.dram_tensor("ident", [128, 128], bf16, kind="ExternalInput").ap()
    out_ap = nc.dram_tensor("out", [BSH, N], f32, kind="ExternalOutput").ap()

    with tile.TileContext(nc) as tc:
        with (
            tc.tile_pool(name="const", bufs=1) as constp,
            tc.tile_pool(name="h0", bufs=8) as h0p,
            tc.tile_pool(name="mstp", bufs=3) as mstp,
            tc.tile_pool(name="chain", bufs=56) as chainp,
            tc.tile_pool(name="tmp11", bufs=6) as tmp11p,
            tc.tile_pool(name="piece", bufs=6) as piecep,
            tc.tile_pool(name="ps1", bufs=5, space="PSUM") as psp1,
            tc.tile_pool(name="ps2", bufs=3, space="PSUM") as psp2,
        ):
            # constants first on the scalar (ACT) HWDGE ring
            tabs = constp.tile([128, T], f32, name="tabs")
            nc.scalar.dma_start(tabs[:], tabs_ap[:])
            tabs16 = constp.tile([128, 2 * T], f32, name="tabs16")
            nc.scalar.dma_start(tabs16[:], tabs16_ap[:])
            idt = constp.tile([128, 128], bf16, name="idt")
            nc.scalar.dma_start(idt[:], id_ap[:])

            def s9col(t):
                return tabs[:, t:t + 1]

            def s10col(t):
                return tabs16[:, t:t + 1]

            def s11col(t):
                return tabs16[:, T + t:T + t + 1]

            # mst chunks: one [128, 4096] DMA per residue class (1MB each)
            mstc = {}
            mstc[0] = mstp.tile([128, 4096], bf16, tag="mst", name="mst_r0")
            nc.scalar.dma_start(mstc[0][:], mst_ap[:, 0:4096])

            # H0 tiles: H0cat[kb][p, lt*512+b] = xT[(4*kb+lt)*128 + p, b]
            H0cat = {}
            for kb in range(8):
                h0c = h0p.tile([128, 2048], bf16, tag="h0", name=f"h0c_{kb}")
                src = xT_ap[kb * 512:(kb + 1) * 512, :].rearrange(
                    "(lt p) b -> p lt b", lt=4, p=128
                )
                nc.sync.dma_start(h0c[:].rearrange("p (lt b) -> p lt b", lt=4), src)
                H0cat[kb] = h0c

            F, P, H11 = {}, {}, {}

            def emit_phase2(r):
                """Transpose H11 tiles of residue r via PE identity matmuls
                into 128-col PSUM quarters; evacuate on ACT; strided store."""
                for bb in range(4):
                    for g2 in range(2):
                        p2 = psp2.tile(
                            [128, 512], f32, tag="ps2", name=f"p2_{r}_{bb}_{g2}"
                        )
                        for i in range(4):
                            d = 4 * (4 * g2 + i) + r
                            nc.tensor.matmul(
                                p2[:, i * 128:(i + 1) * 128],
                                H11[d][:, bb * 128:(bb + 1) * 128],
                                idt[:],
                                start=True, stop=True,
                            )
                        piece = piecep.tile([128, 512], f32, tag="piece")
                        nc.scalar.copy(piece[:], p2[:])
                        # out[bb*128+p, (16*g2+4*i+r)*128 + c] = piece[p, i*128+c]
                        dst = out_ap[bb * 128:(bb + 1) * 128, :].rearrange(
                            "p (mm rr c) -> p mm rr c", mm=8, rr=4, c=128
                        )[:, 4 * g2:4 * g2 + 4, r, :]
                        src = piece[:].rearrange("p (i c) -> p i c", i=4)
                        nc.sync.dma_start(dst, src)

            for r in range(4):
                if r + 1 < 4:
                    mstc[r + 1] = mstp.tile(
                        [128, 4096], bf16, tag="mst", name=f"mst_r{r + 1}"
                    )
                    nc.scalar.dma_start(
                        mstc[r + 1][:],
                        mst_ap[:, (r + 1) * 4096:(r + 2) * 4096],
                    )
                # ---- phase 1 + L9 in partner pairs (m, m^1): evacuate ONE
                # psum of each pair to SBUF (ACT), then both stt ops read
                # one PSUM + one SBUF operand (PSUM has a single DVE port)
                for mp in range(4):
                    p1pair = {}
                    for m in (2 * mp, 2 * mp + 1):
                        t = 4 * m + r
                        p1 = psp1.tile([128, 512], f32, tag="ps1", name=f"p1_{t}")
                        for ji in range(4):
                            nc.tensor.matmul(
                                p1[:],
                                mstc[r][
                                    :, m * 512 + ji * 128:m * 512 + (ji + 1) * 128
                                ],
                                H0cat[m][:, (r ^ ji) * 512:((r ^ ji) + 1) * 512],
                                start=(ji == 0), stop=(ji == 3),
                            )
                        p1pair[m] = p1
                    m0, m1 = 2 * mp, 2 * mp + 1
                    t0, t1 = 4 * m0 + r, 4 * m1 + r
                    ep = chainp.tile([128, 512], bf16, tag="ch", name=f"Ep_{t1}")
                    nc.scalar.copy(ep[:], p1pair[m1][:])
                    F[t0] = chainp.tile([128, 512], bf16, tag="ch", name=f"F_{t0}")
                    nc.vector.scalar_tensor_tensor(
                        F[t0][:], ep[:], s9col(t0), p1pair[m0][:],
                        op0=mult, op1=add,
                    )
                    F[t1] = chainp.tile([128, 512], bf16, tag="ch", name=f"F_{t1}")
                    nc.vector.scalar_tensor_tensor(
                        F[t1][:], p1pair[m0][:], s9col(t1), ep[:],
                        op0=mult, op1=add,
                    )
                # ---- phase 2 of the previous residue (so the PE queue never
                # waits on this residue's DVE chain)
                if r >= 1:
                    emit_phase2(r - 1)
                # ---- L10 (stt, bf16 scalars) and L11 (ts+tt pair)
                for m in range(8):
                    t = 4 * m + r
                    P[t] = chainp.tile([128, 512], bf16, tag="ch", name=f"P_{t}")
                    nc.vector.scalar_tensor_tensor(
                        P[t][:], F[t ^ 8][:], s10col(t), F[t][:],
                        op0=mult, op1=add,
                    )
                for m in range(8):
                    t = 4 * m + r
                    tmp = tmp11p.tile([128, 512], bf16, tag="t11")
                    nc.vector.tensor_scalar_mul(tmp[:], P[t ^ 16][:], s11col(t))
                    H11[t] = chainp.tile([128, 512], bf16, tag="ch", name=f"H11_{t}")
                    eng = nc.gpsimd if m % 2 == 0 else nc.vector
                    eng.tensor_tensor(H11[t][:], tmp[:], P[t][:], op=add)
            emit_phase2(3)

    nc.compile()
    return nc


def kernel(x, values, idx_in, idx_out):
    global LAST_EXEC_NS
    from concourse.bass_utils import run_bass_kernel_spmd

    x = np.asarray(x, np.float32)
    assert x.shape == (BATCH, N), x.shape
    mst, tabs, tabs16, ident = _host_precompute(values, idx_in, idx_out)
    xT = np.ascontiguousarray(x.T.astype(ml_dtypes.bfloat16))

    if "prog" not in _PROGRAM_CACHE:
        _PROGRAM_CACHE["prog"] = _build_program()
    nc = _PROGRAM_CACHE["prog"]

    in_maps = [
        {
            "xT": np.ascontiguousarray(xT[:, i * BSH:(i + 1) * BSH]),
            "mst": mst,
            "tabs": tabs,
            "tabs16": tabs16,
            "ident": ident,
        }
        for i in range(NCORES)
    ]
    res = run_bass_kernel_spmd(nc, in_maps, core_ids=list(range(NCORES)))
    if os.environ.get("BENES_TRACE"):
        tres = run_bass_kernel_spmd(
            nc, in_maps, core_ids=list(range(NCORES)), trace=True
        )
        LAST_EXEC_NS = tres.exec_time_ns
        _PROGRAM_CACHE["profile_json"] = tres.profile_json
    out = np.empty((BATCH, N), np.float32)
    for i in range(NCORES):
        out[i * BSH:(i + 1) * BSH] = res.results[i]["out"]
    return out


# revision 8
# speedup vs baseline: 1.4566x; 1.3502x over previous
"""Benes butterfly network (12 layers, N=4096) on 8 Trainium2 NeuronCores.

Self-contained: takes full inputs, shards batch across 8 cores, runs a
Bass/Tile kernel per core, gathers the full output.

Math: reference layer k is a butterfly with span 2^k:
    h[:, j] <- A_k[j] * h[:, j] + B_k[j] * h[:, j ^ 2^k]
(A_k/B_k extracted from the sparse COO (values, idx_in, idx_out)).

v5 design (DMA-bound baseline at 33.6MB/core cut to 21MB, all wire data
bf16; structure chosen from measured TRN2 op costs — stt 747ns,
tensor_scalar-with-AP-scalar 1232ns, ACT copy 687ns — which rule out
element-wise implementations of more than one butterfly pass):
  - Layers 0..8 composed on the host into dense 128x128 block matrices
    (mst, bf16) with layer 9's self-scale A9 folded in:
      p1[t] = sum_{ji} M[t, t^ji] @ H0[t^ji]     (PE, N=512, fp32 psum)
  - L9 partner via the ratio trick, one DVE stt pass; partner pairs
    (t, t^4) share one ACT evacuation so each stt reads one PSUM + one
    SBUF operand:  H9[t] = p1[t] + (B9[t]/A9[t^4]) * p1[t^4]
  - L10+L11 fused into the PE out-transpose (phase 2): per quad
    {q, q+8, q+16, q+24}, psum[b, 4*128] accumulates 4 matmuls with
    host-built diagonal-block moving matrices (movd, bf16).
  - Residue pipelining: out tile d depends only on tiles t = d (mod 4);
    tiles processed per residue class so residue-0 stores overlap
    residue-1+ loads; emission interleaved (phase1(r+1) ahead of
    phase2(r)) so no engine queue ever stalls on another engine's chain.
"""
import os
import numpy as np
import ml_dtypes

N = 4096
BATCH = 4096
NLAYERS = 12
NCORES = 8
BSH = BATCH // NCORES      # 512 batch rows per core
T = N // 128               # 32 column tiles

_PROGRAM_CACHE = {}
LAST_EXEC_NS = None


def _extract_ab(values, idx_in, idx_out):
    """Per-layer butterfly coefficients A[k], B[k] (float64 [L, N])."""
    v = np.asarray(values, np.float64)
    ii = np.asarray(idx_in, np.int64)
    io = np.asarray(idx_out, np.int64)
    L, nnz = v.shape
    n = nnz // 2
    A = np.zeros((L, n))
    B = np.zeros((L, n))
    for k in range(L):
        s = 1 << k
        self_m = ii[k] == io[k]
        part_m = ii[k] == (io[k] ^ s)
        if not np.all(self_m | part_m):
            raise ValueError(f"layer {k}: unexpected sparse index structure")
        np.add.at(A[k], io[k][self_m], v[k][self_m])
        np.add.at(B[k], io[k][part_m], v[k][part_m])
    return A, B


def _clamp(a):
    return np.where(np.abs(a) < 1e-12, 1e-12, a)


# Residue-major tile ordering: seq[r*8 + m] = 4*m + r
_TSEQ = [4 * m + r for r in range(4) for m in range(8)]


def _host_precompute(values, idx_in, idx_out):
    A, B = _extract_ab(values, idx_in, idx_out)
    Ab = A.reshape(NLAYERS, T, 128)
    Bb = B.reshape(NLAYERS, T, 128)
    j = np.arange(128)

    # Dense composition of within-block layers 0..6, one 128x128 per tile.
    S = [np.eye(128) for _ in range(T)]
    for k in range(7):
        s = 1 << k
        for t in range(T):
            W = np.zeros((128, 128))
            W[j, j] = Ab[k, t]
            W[j, j ^ s] = Bb[k, t]
            S[t] = W @ S[t]
    # Cross-block layers 7, 8 (tile distances 1, 2): dict src_tile -> 128x128
    Sd = [{t: S[t]} for t in range(T)]
    for k in (7, 8):
        d = 1 << (k - 7)
        newS = []
        for t in range(T):
            out = {}
            for src, M in Sd[t].items():
                out[src] = Ab[k, t][:, None] * M
            for src, M in Sd[t ^ d].items():
                out[src] = out.get(src, 0) + Bb[k, t][:, None] * M
            newS.append(out)
        Sd = newS

    A9 = _clamp(Ab[9])

    # mst (bf16, residue-major tile order): block (t, ji) holds
    # (diag(A9[t]) @ Sd[t][t^ji]).T so matmul computes M @ H0.
    mst = np.zeros((128, T * 512), np.float32)
    for si, t in enumerate(_TSEQ):
        assert set(Sd[t].keys()) == {t, t ^ 1, t ^ 2, t ^ 3}
        for ji in range(4):
            M = A9[t][:, None] * Sd[t][t ^ ji]
            mst[:, si * 512 + ji * 128: si * 512 + (ji + 1) * 128] = (
                M.T.astype(np.float32)
            )

    # L9 partner ratio scales rB9[t] = B9[t] / A9[t^4]  (fp32 [128, T])
    tabs = np.zeros((128, T), np.float64)
    for t in range(T):
        tabs[:, t] = Bb[9, t] / A9[t ^ 4]

    # out-transpose movings, quad-major: for quad q, slot si (src s=q+8*si),
    # block k holds diag(c[q+8k <- s]) where c are the fused L10*L11
    # coefficients acting on H9 (post-L9 state)
    movd = np.zeros((128, T * 512), np.float32)
    for s in range(T):
        q = s & 7
        si = s >> 3
        for k in range(4):
            d = q + 8 * k
            if s == d:
                c = Ab[11, d] * Ab[10, d]
            elif s == (d ^ 8):
                c = Ab[11, d] * Bb[10, d]
            elif s == (d ^ 16):
                c = Bb[11, d] * Ab[10, d ^ 16]
            else:  # s == d ^ 24
                c = Bb[11, d] * Bb[10, d ^ 16]
            movd[j, q * 2048 + si * 512 + k * 128 + j] = c.astype(np.float32)

    return (
        mst.astype(ml_dtypes.bfloat16),
        tabs.astype(np.float32),
        movd.astype(ml_dtypes.bfloat16),
    )


def _build_program():
    import concourse.bass as bass
    import concourse.mybir as mybir
    import concourse.tile as tile
    from concourse import bacc

    f32 = mybir.dt.float32
    bf16 = mybir.dt.bfloat16
    mult = mybir.AluOpType.mult
    add = mybir.AluOpType.add

    nc = bacc.Bacc("TRN2", target_bir_lowering=False, debug=False)
    xT_ap = nc.dram_tensor("xT", [N, BSH], bf16, kind="ExternalInput").ap()
    mst_ap = nc.dram_tensor("mst", [128, T * 512], bf16, kind="ExternalInput").ap()
    tabs_ap = nc.dram_tensor("tabs", [128, T], f32, kind="ExternalInput").ap()
    mov_ap = nc.dram_tensor("movd", [128, T * 512], bf16, kind="ExternalInput").ap()
    out_ap = nc.dram_tensor("out", [BSH, N], f32, kind="ExternalOutput").ap()

    with tile.TileContext(nc) as tc:
        with (
            tc.tile_pool(name="const", bufs=1) as constp,
            tc.tile_pool(name="h0", bufs=8) as h0p,
            tc.tile_pool(name="mstp", bufs=3) as mstp,
            tc.tile_pool(name="chain", bufs=56) as chainp,
            tc.tile_pool(name="mov", bufs=4) as movp,
            tc.tile_pool(name="piece", bufs=6) as piecep,
            tc.tile_pool(name="ps1", bufs=5, space="PSUM") as psp1,
            tc.tile_pool(name="ps2", bufs=3, space="PSUM") as psp2,
        ):
            tabs = constp.tile([128, T], f32, name="tabs")
            nc.scalar.dma_start(tabs[:], tabs_ap[:])

            def rb9col(t):
                return tabs[:, t:t + 1]

            # mst chunks: one [128, 4096] DMA per residue class (1MB each)
            mstc = {}
            mstc[0] = mstp.tile([128, 4096], bf16, tag="mst", name="mst_r0")
            nc.scalar.dma_start(mstc[0][:], mst_ap[:, 0:4096])

            # H0 tiles: H0cat[kb][p, lt*512+b] = xT[(4*kb+lt)*128 + p, b]
            H0cat = {}
            for kb in range(8):
                h0c = h0p.tile([128, 2048], bf16, tag="h0", name=f"h0c_{kb}")
                src = xT_ap[kb * 512:(kb + 1) * 512, :].rearrange(
                    "(lt p) b -> p lt b", lt=4, p=128
                )
                nc.sync.dma_start(h0c[:].rearrange("p (lt b) -> p lt b", lt=4), src)
                H0cat[kb] = h0c

            H9 = {}
            mv = {}

            def emit_phase2(r):
                """L10+L11 + out-transpose for the two quads of residue r."""
                for q in (r, r + 4):
                    for bb in range(4):
                        pq = psp2.tile(
                            [128, 512], f32, tag="ps2", name=f"pq_{q}_{bb}"
                        )
                        for si in range(4):
                            s = q + 8 * si
                            nc.tensor.matmul(
                                pq[:], H9[s][:, bb * 128:(bb + 1) * 128],
                                mv[q][:, si * 512:(si + 1) * 512],
                                start=(si == 0), stop=(si == 3),
                            )
                        piece = piecep.tile([128, 512], f32, tag="piece")
                        if (q + bb) % 2 == 0:
                            nc.scalar.copy(piece[:], pq[:])
                        else:
                            nc.vector.tensor_copy(piece[:], pq[:])
                        dst = out_ap[bb * 128:(bb + 1) * 128, :].rearrange(
                            "p (k t c) -> p k t c", k=4, t=8, c=128
                        )[:, :, q, :]
                        src = piece[:].rearrange("p (k c) -> p k c", k=4, c=128)
                        nc.sync.dma_start(dst, src)

            for r in range(4):
                # prefetch next residue's mst chunk and this residue's movd
                # quad chunks (used by phase2(r) during phase1(r+1))
                if r + 1 < 4:
                    mstc[r + 1] = mstp.tile(
                        [128, 4096], bf16, tag="mst", name=f"mst_r{r + 1}"
                    )
                    nc.scalar.dma_start(
                        mstc[r + 1][:],
                        mst_ap[:, (r + 1) * 4096:(r + 2) * 4096],
                    )
                for q in (r, r + 4):
                    mv[q] = movp.tile([128, 2048], bf16, tag="mov", name=f"mv_{q}")
                    nc.scalar.dma_start(
                        mv[q][:], mov_ap[:, q * 2048:(q + 1) * 2048]
                    )
                # ---- phase 1 + L9 in partner pairs (m, m^1): evacuate ONE
                # psum of each pair to SBUF (ACT), then both stt ops read
                # one PSUM + one SBUF operand (PSUM has a single DVE port)
                for mp in range(4):
                    p1pair = {}
                    for m in (2 * mp, 2 * mp + 1):
                        t = 4 * m + r
                        p1 = psp1.tile([128, 512], f32, tag="ps1", name=f"p1_{t}")
                        for ji in range(4):
                            nc.tensor.matmul(
                                p1[:],
                                mstc[r][
                                    :, m * 512 + ji * 128:m * 512 + (ji + 1) * 128
                                ],
                                H0cat[m][:, (r ^ ji) * 512:((r ^ ji) + 1) * 512],
                                start=(ji == 0), stop=(ji == 3),
                            )
                        p1pair[m] = p1
                    m0, m1 = 2 * mp, 2 * mp + 1
                    t0, t1 = 4 * m0 + r, 4 * m1 + r
                    ep = chainp.tile([128, 512], bf16, tag="ch", name=f"Ep_{t1}")
                    nc.scalar.copy(ep[:], p1pair[m1][:])
                    H9[t0] = chainp.tile(
                        [128, 512], bf16, tag="ch", name=f"H9_{t0}"
                    )
                    nc.vector.scalar_tensor_tensor(
                        H9[t0][:], ep[:], rb9col(t0), p1pair[m0][:],
                        op0=mult, op1=add,
                    )
                    H9[t1] = chainp.tile(
                        [128, 512], bf16, tag="ch", name=f"H9_{t1}"
                    )
                    nc.vector.scalar_tensor_tensor(
                        H9[t1][:], p1pair[m0][:], rb9col(t1), ep[:],
                        op0=mult, op1=add,
                    )
                # ---- phase 2 of the previous residue (so the PE queue never
                # waits on this residue's stt pass)
                if r >= 1:
                    emit_phase2(r - 1)
            emit_phase2(3)

    nc.compile()
    return nc


def kernel(x, values, idx_in, idx_out):
    global LAST_EXEC_NS
    from concourse.bass_utils import run_bass_kernel_spmd

    x = np.asarray(x, np.float32)
    assert x.shape == (BATCH, N), x.shape
    mst, tabs, movd = _host_precompute(values, idx_in, idx_out)
    xT = np.ascontiguousarray(x.T.astype(ml_dtypes.bfloat16))

    if "prog" not in _PROGRAM_CACHE:
        _PROGRAM_CACHE["prog"] = _build_program()
    nc = _PROGRAM_CACHE["prog"]

    in_maps = [
        {
            "xT": np.ascontiguousarray(xT[:, i * BSH:(i + 1) * BSH]),
            "mst": mst,
            "tabs": tabs,
            "movd": movd,
        }
        for i in range(NCORES)
    ]
    res = run_bass_kernel_spmd(nc, in_maps, core_ids=list(range(NCORES)))
    if os.environ.get("BENES_TRACE"):
        tres = run_bass_kernel_spmd(
            nc, in_maps, core_ids=list(range(NCORES)), trace=True
        )
        LAST_EXEC_NS = tres.exec_time_ns
        _PROGRAM_CACHE["profile_json"] = tres.profile_json
    out = np.empty((BATCH, N), np.float32)
    for i in range(NCORES):
        out[i * BSH:(i + 1) * BSH] = res.results[i]["out"]
    return out
